# revision 4
# baseline (speedup 1.0000x reference)
"""VQ codebook squared-distance kernel for Trainium2 (8 NeuronCores).

Computes dist[n,k,l] = (||x[n,:,l]||^2 + ||w[k,:]||^2 - 2*x[n,:,l].w[k,:]) / scale^2
for x (32,128,3136) f32, weight (64,128) f32, scale (1,) f32 -> out (32,64,3136) f32.

Sharding: data-parallel over N (4 per core); weight/scale replicated.

Per-core kernel structure:
  - precompute  Wt2 = -2/s^2 * W^T          (128p x 64f)   [PE transpose]
                ones_sc = 1/s^2 * ones      (128p x 64f)
                bias = ||w_k||^2 / s^2      (128p x 1) (k repeated on both halves)
  - per n:      DMA x[n] (128,3136); ACT square -> x^2
  - per (pair of n, chunk of 448 l):
        psum[0:64]   = Wt2^T @ x[2p]   + ones_sc^T @ x^2[2p]     (col group 0-1)
        psum[64:128] = Wt2^T @ x[2p+1] + ones_sc^T @ x^2[2p+1]   (col group 2-3)
        out_sbuf[:, chunk] = psum + bias                          (DVE)
  - per pair:   DMA out_sbuf (128,3136) -> dist[2p:2p+2]  (full-partition store)
"""

import numpy as np

N, D, L, K = 32, 128, 3136, 64
N_CORES = 8
NS = N // N_CORES          # n's per core
LC = 448                   # l-chunk (fits one PSUM bank: 448*4B < 2KB)
NCHUNK = L // LC           # 7

_cache = {}


def _build():
    import concourse.bacc as bacc
    import concourse.mybir as mybir
    import concourse.tile as tile
    from concourse.masks import make_identity

    f32 = mybir.dt.float32
    AF = mybir.ActivationFunctionType

    nc = bacc.Bacc(
        "TRN2",
        target_bir_lowering=False,
        debug=False,
        enable_asserts=False,
        num_devices=N_CORES,
    )

    x_ap = nc.dram_tensor("x", (NS, D, L), f32, kind="ExternalInput").ap()
    w_ap = nc.dram_tensor("weight", (K, D), f32, kind="ExternalInput").ap()
    s_ap = nc.dram_tensor("scale", (1,), f32, kind="ExternalInput").ap()
    o_ap = nc.dram_tensor("out", (NS, K, L), f32, kind="ExternalOutput").ap()

    with tile.TileContext(nc) as tc:
        with (
            tc.tile_pool(name="consts", bufs=1) as consts,
            tc.tile_pool(name="xin", bufs=4) as xpool,
            tc.tile_pool(name="xsq", bufs=3) as xqpool,
            tc.tile_pool(name="outp", bufs=2) as opool,
            tc.tile_pool(name="psum", bufs=4, space="PSUM") as pspool,
            tc.tile_pool(name="psum1", bufs=1, space="PSUM") as pspool1,
        ):
            # ---- constants -------------------------------------------------
            # weight replicated onto both partition halves
            w2 = consts.tile([2 * K, D], f32)
            nc.sync.dma_start(out=w2[0:K, :], in_=w_ap)
            nc.sync.dma_start(out=w2[K : 2 * K, :], in_=w_ap)

            # scale broadcast to all 128 partitions; inv_s2 = 1/scale^2
            s_b = consts.tile([128, 1], f32)
            nc.gpsimd.dma_start(out=s_b, in_=s_ap.to_broadcast((128, 1)))
            inv_s2 = consts.tile([128, 1], f32)
            nc.vector.tensor_mul(inv_s2, s_b, s_b)
            nc.vector.reciprocal(inv_s2, inv_s2)
            neg2inv = consts.tile([128, 1], f32)
            nc.scalar.mul(neg2inv, inv_s2, -2.0)

            # W^T via PE transpose
            ident = consts.tile([K, K], f32)
            make_identity(nc, ident)
            ps_w = pspool1.tile([D, K], f32)
            nc.tensor.transpose(ps_w, w2[0:K, :], ident)
            wT2 = consts.tile([D, K], f32)
            nc.vector.tensor_scalar_mul(wT2, in0=ps_w, scalar1=neg2inv)

            # ones * inv_s2
            ones_sc = consts.tile([D, K], f32)
            nc.vector.memset(ones_sc, 1.0)
            nc.vector.tensor_scalar_mul(ones_sc, in0=ones_sc, scalar1=inv_s2)

            # bias = ||w_k||^2 * inv_s2, on both partition halves
            w_sq = consts.tile([2 * K, D], f32)
            nc.vector.tensor_mul(w_sq, w2, w2)
            bias = consts.tile([2 * K, 1], f32)
            nc.vector.reduce_sum(out=bias, in_=w_sq, axis=mybir.AxisListType.X)
            nc.vector.tensor_mul(bias, bias, inv_s2)

            # ---- main loop -------------------------------------------------
            for pair in range(NS // 2):
                xs = []
                for s in range(2):
                    n = 2 * pair + s
                    xt = xpool.tile([D, L], f32, tag="xt", name=f"x_{n}")
                    nc.sync.dma_start(out=xt, in_=x_ap[n])
                    xq = xqpool.tile([D, L], f32, tag="xq", name=f"xsq_{n}")
                    nc.scalar.activation(xq, xt, AF.Square)
                    xs.append((xt, xq))

                out_t = opool.tile([2 * K, L], f32, tag="out_t", name=f"out_{pair}")
                for c in range(NCHUNK):
                    sl = slice(c * LC, (c + 1) * LC)
                    ps = pspool.tile([2 * K, LC], f32, name="ps")
                    nc.tensor.matmul(
                        ps[0:K, :], wT2, xs[0][0][:, sl],
                        start=True, stop=False, tile_position=(0, 0),
                    )
                    nc.tensor.matmul(
                        ps[K : 2 * K, :], wT2, xs[1][0][:, sl],
                        start=True, stop=False, tile_position=(0, 64),
                    )
                    nc.tensor.matmul(
                        ps[0:K, :], ones_sc, xs[0][1][:, sl],
                        start=False, stop=True, tile_position=(0, 0),
                    )
                    nc.tensor.matmul(
                        ps[K : 2 * K, :], ones_sc, xs[1][1][:, sl],
                        start=False, stop=True, tile_position=(0, 64),
                    )
                    nc.vector.tensor_scalar_add(
                        out=out_t[:, sl], in0=ps, scalar1=bias
                    )
                o_pair = o_ap[2 * pair : 2 * pair + 2].rearrange("a k l -> (a k) l")
                nc.sync.dma_start(out=o_pair, in_=out_t)

    nc.compile()
    return nc


def _get_nc():
    if "nc" not in _cache:
        _cache["nc"] = _build()
    return _cache["nc"]


def run(x, weight, scale, trace=False):
    from concourse.bass_utils import run_bass_kernel_spmd

    x = np.ascontiguousarray(np.asarray(x, dtype=np.float32))
    weight = np.ascontiguousarray(np.asarray(weight, dtype=np.float32))
    scale = np.ascontiguousarray(np.asarray(scale, dtype=np.float32))
    assert x.shape == (N, D, L) and weight.shape == (K, D) and scale.shape == (1,)

    nc = _get_nc()
    in_maps = [
        {"x": x[c * NS : (c + 1) * NS], "weight": weight, "scale": scale}
        for c in range(N_CORES)
    ]
    res = run_bass_kernel_spmd(
        nc, in_maps, core_ids=list(range(N_CORES)), trace=trace
    )
    out = np.concatenate([r["out"] for r in res.results], axis=0)
    return out, res


def kernel(x, weight, scale):
    out, _ = run(x, weight, scale, trace=False)
    return out


# revision 13
# speedup vs baseline: 1.0711x; 1.0711x over previous
"""VQ codebook squared-distance kernel for Trainium2 (8 NeuronCores).

Computes dist[n,k,l] = (||x[n,:,l]||^2 + ||w[k,:]||^2 - 2*x[n,:,l].w[k,:]) / scale^2
for x (32,128,3136) f32, weight (64,128) f32, scale (1,) f32 -> out (32,64,3136) f32.

Sharding: data-parallel over N (4 per core); weight/scale replicated.

Per-core kernel structure:
  - precompute  Wt2 = -2/s^2 * W^T          (128p x 64f)   [PE transpose]
                ones_sc = 1/s^2 * ones      (128p x 64f)
                bias = ||w_k||^2 / s^2      (128p x 1) (k repeated on both halves)
  - per n:      DMA x[n] (128,3136); ACT square -> x^2
  - per (pair of n, chunk of 448 l):
        psum[0:64]   = Wt2^T @ x[2p]   + ones_sc^T @ x^2[2p]     (col group 0-1)
        psum[64:128] = Wt2^T @ x[2p+1] + ones_sc^T @ x^2[2p+1]   (col group 2-3)
        out_sbuf[:, chunk] = psum + bias                          (DVE)
  - per pair:   DMA out_sbuf (128,3136) -> dist[2p:2p+2]  (full-partition store)
"""

import numpy as np

N, D, L, K = 32, 128, 3136, 64
N_CORES = 8
NS = N // N_CORES          # n's per core
LC = 448                   # l-chunk (fits one PSUM bank: 448*4B < 2KB)
NCHUNK = L // LC           # 7

_cache = {}


def _build():
    import concourse.bacc as bacc
    import concourse.mybir as mybir
    import concourse.tile as tile
    from concourse.masks import make_identity

    f32 = mybir.dt.float32
    f32r = mybir.dt.float32r
    AF = mybir.ActivationFunctionType

    nc = bacc.Bacc(
        "TRN2",
        target_bir_lowering=False,
        debug=False,
        enable_asserts=False,
        num_devices=N_CORES,
    )

    x_ap = nc.dram_tensor("x", (NS, D, L), f32r, kind="ExternalInput").ap()
    w_ap = nc.dram_tensor("weight", (K, D), f32, kind="ExternalInput").ap()
    s_ap = nc.dram_tensor("scale", (1,), f32, kind="ExternalInput").ap()
    o_ap = nc.dram_tensor("out", (NS, K, L), f32, kind="ExternalOutput").ap()

    with tile.TileContext(nc) as tc:
        with (
            tc.tile_pool(name="consts", bufs=1) as consts,
            tc.tile_pool(name="xin", bufs=4) as xpool,
            tc.tile_pool(name="xsq", bufs=3) as xqpool,
            tc.tile_pool(name="outp", bufs=2) as opool,
            tc.tile_pool(name="psum", bufs=4, space="PSUM") as pspool,
            tc.tile_pool(name="psum1", bufs=1, space="PSUM") as pspool1,
        ):
            # ---- constants -------------------------------------------------
            # weight replicated onto both partition halves
            w2 = consts.tile([2 * K, D], f32)
            nc.sync.dma_start(out=w2[0:K, :], in_=w_ap)
            nc.sync.dma_start(out=w2[K : 2 * K, :], in_=w_ap)

            # scale broadcast to all 128 partitions; inv_s2 = 1/scale^2
            s_b = consts.tile([128, 1], f32)
            nc.gpsimd.dma_start(out=s_b, in_=s_ap.to_broadcast((128, 1)))
            inv_s2 = consts.tile([128, 1], f32)
            nc.vector.tensor_mul(inv_s2, s_b, s_b)
            nc.vector.reciprocal(inv_s2, inv_s2)
            neg2inv = consts.tile([128, 1], f32)
            nc.scalar.mul(neg2inv, inv_s2, -2.0)

            # W^T via PE transpose
            ident = consts.tile([K, K], f32)
            make_identity(nc, ident)
            ps_w = pspool1.tile([D, K], f32)
            nc.tensor.transpose(ps_w, w2[0:K, :], ident)

            # Zero-padded 128-wide stationary operands: even-n data in array
            # columns 0-63, odd-n data in columns 64-127.  Each matmul then
            # writes the full 128-partition PSUM (other half accumulates +0),
            # so a pair of n's shares one PSUM tile / one full-width output.
            ones_f = consts.tile([D, K], f32)
            nc.vector.memset(ones_f, 1.0)
            wT_pair = []
            ones_pair = []
            for s in range(2):
                tmp_w = consts.tile([D, 2 * K], f32, name=f"tmpw_{s}")
                nc.vector.memset(tmp_w, 0.0)
                nc.vector.tensor_scalar_mul(
                    tmp_w[:, s * K : (s + 1) * K], in0=ps_w, scalar1=neg2inv
                )
                wTs = consts.tile([D, 2 * K], f32r, name=f"wT_{s}")
                nc.vector.tensor_copy(wTs, tmp_w)
                wT_pair.append(wTs)

                tmp_o = consts.tile([D, 2 * K], f32, name=f"tmpo_{s}")
                nc.vector.memset(tmp_o, 0.0)
                nc.vector.tensor_scalar_mul(
                    tmp_o[:, s * K : (s + 1) * K], in0=ones_f, scalar1=inv_s2
                )
                ons = consts.tile([D, 2 * K], f32r, name=f"ones_{s}")
                nc.vector.tensor_copy(ons, tmp_o)
                ones_pair.append(ons)

            # bias = ||w_k||^2 * inv_s2, on both partition halves
            w_sq = consts.tile([2 * K, D], f32)
            nc.vector.tensor_mul(w_sq, w2, w2)
            bias = consts.tile([2 * K, 1], f32)
            nc.vector.reduce_sum(out=bias, in_=w_sq, axis=mybir.AxisListType.X)
            nc.vector.tensor_mul(bias, bias, inv_s2)

            # ---- main loop -------------------------------------------------
            for pair in range(NS // 2):
                xs = []
                for s in range(2):
                    n = 2 * pair + s
                    xt = xpool.tile([D, L], f32r, tag="xt", name=f"x_{n}")
                    nc.sync.dma_start(out=xt, in_=x_ap[n])
                    xq = xqpool.tile([D, L], f32r, tag="xq", name=f"xsq_{n}")
                    nc.scalar.activation(xq, xt.bitcast(f32), AF.Square)
                    xs.append((xt, xq))

                out_t = opool.tile([2 * K, L], f32, tag="out_t", name=f"out_{pair}")
                for c in range(NCHUNK):
                    sl = slice(c * LC, (c + 1) * LC)
                    ps = pspool.tile([2 * K, LC], f32, name="ps")
                    nc.tensor.matmul(
                        ps, wT_pair[0], xs[0][0][:, sl], start=True, stop=False
                    )
                    nc.tensor.matmul(
                        ps, wT_pair[1], xs[1][0][:, sl], start=False, stop=False
                    )
                    nc.tensor.matmul(
                        ps, ones_pair[0], xs[0][1][:, sl], start=False, stop=False
                    )
                    nc.tensor.matmul(
                        ps, ones_pair[1], xs[1][1][:, sl], start=False, stop=True
                    )
                    nc.vector.tensor_scalar_add(
                        out=out_t[:, sl], in0=ps, scalar1=bias
                    )
                o_pair = o_ap[2 * pair : 2 * pair + 2].rearrange("a k l -> (a k) l")
                # output stores ride the ACT HWDGE ring so input loads (sync
                # ring) and stores use disjoint SDMA engine sets
                nc.scalar.dma_start(out=o_pair, in_=out_t)

    nc.compile()
    return nc


def _get_nc():
    if "nc" not in _cache:
        _cache["nc"] = _build()
    return _cache["nc"]


def run(x, weight, scale, trace=False):
    from concourse.bass_utils import run_bass_kernel_spmd

    x = np.ascontiguousarray(np.asarray(x, dtype=np.float32))
    weight = np.ascontiguousarray(np.asarray(weight, dtype=np.float32))
    scale = np.ascontiguousarray(np.asarray(scale, dtype=np.float32))
    assert x.shape == (N, D, L) and weight.shape == (K, D) and scale.shape == (1,)

    nc = _get_nc()
    in_maps = [
        {"x": x[c * NS : (c + 1) * NS], "weight": weight, "scale": scale}
        for c in range(N_CORES)
    ]
    res = run_bass_kernel_spmd(
        nc, in_maps, core_ids=list(range(N_CORES)), trace=trace
    )
    out = np.concatenate([r["out"] for r in res.results], axis=0)
    return out, res


def kernel(x, weight, scale):
    out, _ = run(x, weight, scale, trace=False)
    return out


# revision 14
# speedup vs baseline: 1.3278x; 1.2397x over previous
"""VQ codebook squared-distance kernel for Trainium2 (8 NeuronCores).

Computes dist[n,k,l] = (||x[n,:,l]||^2 + ||w[k,:]||^2 - 2*x[n,:,l].w[k,:]) / scale^2
for x (32,128,3136) f32, weight (64,128) f32, scale (1,) f32 -> out (32,64,3136) f32.

Sharding: data-parallel over N (4 per core); weight/scale replicated.

Per-core design (fp16 PE path):
  - x is DMA-cast fp32->fp16 on load (SWDGE); squares computed on ACT in fp16.
  - PE: psum[k(2),l] = (-2Wt)fp16 @ x_fp16  +  ones_fp16 @ (x^2)_fp16,
    two n's packed per PSUM tile via column tiling (tile_position (0,0)/(0,64));
    contraction D=128, fp32 PSUM accumulate.
  - DVE epilogue: out = (psum + ||c_k||^2) * (1/scale^2) in fp32.
  - Output pairs form full 128-partition tiles; stores alternate between the
    two HWDGE rings (sync / scalar) to engage both SDMA engine sets.
"""

import numpy as np

N, D, L, K = 32, 128, 3136, 64
N_CORES = 8
NS = N // N_CORES          # n's per core
LC = 392                   # l-chunk (8 chunks; 4 per half-tile of 1568)
NCHUNK = L // LC           # 8
LH = L // 2                # half length for DMA/square granularity

_cache = {}


def _build():
    import concourse.bacc as bacc
    import concourse.mybir as mybir
    import concourse.tile as tile
    from concourse.masks import make_identity

    f32 = mybir.dt.float32
    f16 = mybir.dt.float16
    AF = mybir.ActivationFunctionType

    nc = bacc.Bacc(
        "TRN2",
        target_bir_lowering=False,
        debug=False,
        enable_asserts=False,
        num_devices=N_CORES,
    )

    x_ap = nc.dram_tensor("x", (NS, D, L), f32, kind="ExternalInput").ap()
    w_ap = nc.dram_tensor("weight", (K, D), f32, kind="ExternalInput").ap()
    s_ap = nc.dram_tensor("scale", (1,), f32, kind="ExternalInput").ap()
    o_ap = nc.dram_tensor("out", (NS, K, L), f32, kind="ExternalOutput").ap()

    with tile.TileContext(nc) as tc:
        with (
            tc.tile_pool(name="consts", bufs=1) as consts,
            tc.tile_pool(name="xin", bufs=4) as xpool,
            tc.tile_pool(name="xsq", bufs=3) as xqpool,
            tc.tile_pool(name="outp", bufs=2) as opool,
            tc.tile_pool(name="psum", bufs=6, space="PSUM") as pspool,
            tc.tile_pool(name="psum1", bufs=1, space="PSUM") as pspool1,
        ):
            # ---- constants -------------------------------------------------
            # weight replicated onto both partition halves (for c_sq on 128p)
            w2 = consts.tile([2 * K, D], f32)
            nc.sync.dma_start(out=w2[0:K, :], in_=w_ap)
            nc.sync.dma_start(out=w2[K : 2 * K, :], in_=w_ap)

            # scale broadcast to all 128 partitions; inv_s2 = 1/scale^2
            s_b = consts.tile([128, 1], f32)
            nc.gpsimd.dma_start(out=s_b, in_=s_ap.to_broadcast((128, 1)))
            inv_s2 = consts.tile([128, 1], f32)
            nc.vector.tensor_mul(inv_s2, s_b, s_b)
            nc.vector.reciprocal(inv_s2, inv_s2)

            # -2 * W^T in fp16 via PE transpose (single rounding)
            ident = consts.tile([K, K], f32)
            make_identity(nc, ident)
            ps_w = pspool1.tile([D, K], f32)
            nc.tensor.transpose(ps_w, w2[0:K, :], ident)
            wT16 = consts.tile([D, K], f16)
            nc.vector.tensor_scalar_mul(wT16, in0=ps_w, scalar1=-2.0)

            ones16 = consts.tile([D, K], f16)
            nc.vector.memset(ones16, 1.0)

            # c_sq = ||w_k||^2 (fp32, both partition halves)
            w_sq = consts.tile([2 * K, D], f32)
            nc.vector.tensor_mul(w_sq, w2, w2)
            c_sq = consts.tile([2 * K, 1], f32)
            nc.vector.reduce_sum(out=c_sq, in_=w_sq, axis=mybir.AxisListType.X)

            # ---- main loop -------------------------------------------------
            out_ring = [nc.sync, nc.scalar]
            ring_i = 0
            for pair in range(NS // 2):
                xs = []
                for s in range(2):
                    n = 2 * pair + s
                    xt = xpool.tile([D, L], f16, tag="xt", name=f"x_{n}")
                    xq = xqpool.tile([D, L], f16, tag="xq", name=f"xsq_{n}")
                    for h in range(2):
                        hs = slice(h * LH, (h + 1) * LH)
                        nc.gpsimd.dma_start(out=xt[:, hs], in_=x_ap[n][:, hs])
                        nc.scalar.activation(xq[:, hs], xt[:, hs], AF.Square)
                    xs.append((xt, xq))

                out_t = opool.tile([2 * K, L], f32, tag="out_t", name=f"out_{pair}")
                for c in range(NCHUNK):
                    sl = slice(c * LC, (c + 1) * LC)
                    ps = pspool.tile([2 * K, LC], f32, name="ps")
                    nc.tensor.matmul(
                        ps[0:K, :], wT16, xs[0][0][:, sl],
                        start=True, stop=False, tile_position=(0, 0),
                    )
                    nc.tensor.matmul(
                        ps[K : 2 * K, :], wT16, xs[1][0][:, sl],
                        start=True, stop=False, tile_position=(0, 64),
                    )
                    nc.tensor.matmul(
                        ps[0:K, :], ones16, xs[0][1][:, sl],
                        start=False, stop=True, tile_position=(0, 0),
                    )
                    nc.tensor.matmul(
                        ps[K : 2 * K, :], ones16, xs[1][1][:, sl],
                        start=False, stop=True, tile_position=(0, 64),
                    )
                    nc.vector.tensor_scalar(
                        out=out_t[:, sl], in0=ps,
                        scalar1=c_sq, scalar2=inv_s2,
                        op0=mybir.AluOpType.add, op1=mybir.AluOpType.mult,
                    )
                o_pair = o_ap[2 * pair : 2 * pair + 2].rearrange("a k l -> (a k) l")
                for h in range(2):
                    hs = slice(h * LH, (h + 1) * LH)
                    out_ring[ring_i % 2].dma_start(
                        out=o_pair[:, hs], in_=out_t[:, hs]
                    )
                    ring_i += 1

    nc.compile()
    return nc


def _get_nc():
    if "nc" not in _cache:
        _cache["nc"] = _build()
    return _cache["nc"]


def run(x, weight, scale, trace=False):
    from concourse.bass_utils import run_bass_kernel_spmd

    x = np.ascontiguousarray(np.asarray(x, dtype=np.float32))
    weight = np.ascontiguousarray(np.asarray(weight, dtype=np.float32))
    scale = np.ascontiguousarray(np.asarray(scale, dtype=np.float32))
    assert x.shape == (N, D, L) and weight.shape == (K, D) and scale.shape == (1,)

    nc = _get_nc()
    in_maps = [
        {"x": x[c * NS : (c + 1) * NS], "weight": weight, "scale": scale}
        for c in range(N_CORES)
    ]
    res = run_bass_kernel_spmd(
        nc, in_maps, core_ids=list(range(N_CORES)), trace=trace
    )
    out = np.concatenate([r["out"] for r in res.results], axis=0)
    return out, res


def kernel(x, weight, scale):
    out, _ = run(x, weight, scale, trace=False)
    return out
